# revision 4
# baseline (speedup 1.0000x reference)
"""Trainium2 Bass kernel for nn_CustomDropout: per-head attention-distance
dropout.

Reference semantics:
  p[h]   = 0.5 / exp(5 * avgdist[h] / sqrt(N)),
  avgdist[h] = (sum_{b,i,j} attn[b,h,i,j] * dist[i,j]) / (sum attn[b,h,:,:])
  out[b,n,h*Ch+ch] = where(u[h,b,n,ch] >= p[h], x[b,n,c]/(1-p[h]), 0)
  with u = jax.random.uniform(key(42), [H,B,N,Ch]) (input-independent).

Plan (8 NeuronCores, data-parallel over B, 4 batches/core):
  Phase A (bass kernel): per (head) accumulate the 4 local batch slices of
    attn into one [128, 2601] SBUF buffer using SWDGE accumulate-DMA (adds
    happen in the DMA engine), then a fused DVE tensor_tensor_reduce
    (x dist, add-reduce) for the numerator and tensor_reduce for the
    denominator, chunked for fp32 accuracy -> per-partition partial sums.
  Host: combine partials over cores/partitions/chunks in float64, add the
    straggler element (577^2 = 128*2601 + 1; dist there is 0), compute p.
  Phase B (bass kernel): elementwise dropout: mask = (u >= p_c),
    out = mask * (x * 1/(1-p_c)), tiled [128 rows, 768].
"""

import numpy as np

B, H, N, C = 32, 12, 577, 768
Ch = C // H  # 64
N_CORES = 8
B_LOC = B // N_CORES  # 4
NN = N * N  # 332929
P = 128
F = 2601  # NN // 128
NN_MAIN = P * F  # 332928; one straggler element (i=j=576) handled on host
SIDE = N ** 0.5
ROWS = B_LOC * N  # 2308 rows per core in phase B
CHUNKS = [(0, 867), (867, 1734), (1734, 2601)]
NCH = len(CHUNKS)

_STATE = {}


def _build_consts():
    """dist matrix and uniform draws, computed with the exact same jax ops
    (and backend) as the reference so they match bitwise."""
    import jax
    import jax.numpy as jnp

    idx = jnp.arange(N, dtype=jnp.float32)
    d = idx[None, :] - idx[:, None]
    dist = jnp.sqrt((d % SIDE) ** 2 + jnp.floor_divide(d, SIDE) ** 2)
    dist = np.asarray(dist, dtype=np.float32)
    assert dist.shape == (N, N)

    u = jax.random.uniform(
        jax.random.key(42), (H, B, N, Ch), dtype=jnp.float32
    )
    u = np.asarray(u)
    # u_perm[b, n, h*Ch+ch] = u[h, b, n, ch]
    u_perm = np.ascontiguousarray(u.transpose(1, 2, 0, 3)).reshape(B, N, C)

    dist_flat = np.ascontiguousarray(dist.reshape(-1)[:NN_MAIN]).reshape(P, F)
    # the dropped element is (i=576, j=576): d=0 -> dist 0, so it only
    # affects the denominator (added on host)
    assert dist.reshape(-1)[NN_MAIN] == 0.0
    return dist_flat, u_perm


def _build_phase_a():
    import concourse.bacc as bacc
    import concourse.mybir as mybir
    import concourse.tile as tile

    f32 = mybir.dt.float32
    nc = bacc.Bacc("TRN2", target_bir_lowering=False, debug=False,
                   num_devices=N_CORES)
    attn_t = nc.dram_tensor("attn_s", [B_LOC * H, NN], f32,
                            kind="ExternalInput")
    dist_t = nc.dram_tensor("dist", [P, F], f32, kind="ExternalInput")
    part_t = nc.dram_tensor("partials", [P, 2 * NCH * H], f32,
                            kind="ExternalOutput")

    with tile.TileContext(nc) as tc:
        with (
            tc.tile_pool(name="const", bufs=1) as constp,
            tc.tile_pool(name="acc", bufs=H) as accp,
            tc.tile_pool(name="scr", bufs=3) as scrp,
            tc.tile_pool(name="outp", bufs=1) as outp,
        ):
            dist = constp.tile([P, F], f32)
            nc.sync.dma_start(dist[:], dist_t.ap())
            parts = outp.tile([P, 2 * NCH * H], f32)

            accs = []
            for h in range(H):
                accs.append(accp.tile([P, F], f32, tag="acc", name=f"acc{h}"))
            # issue DMAs batch-major so the 12 per-head accumulate chains
            # overlap instead of serializing on the gpsimd queue
            for b in range(B_LOC):
                for h in range(H):
                    src = attn_t.ap()[b * H + h, 0:NN_MAIN].rearrange(
                        "(p f) -> p f", p=P)
                    if b == 0:
                        nc.gpsimd.dma_start(accs[h][:], src)
                    else:
                        # CCE accumulate breaks past 2048 elements per
                        # partition; split the accumulating DMAs
                        for lo, hi in ((0, 1301), (1301, F)):
                            nc.gpsimd.dma_start(
                                accs[h][:, lo:hi], src[:, lo:hi],
                                accum_op=mybir.AluOpType.add)
            for h in range(H):
                for c, (lo, hi) in enumerate(CHUNKS):
                    w = hi - lo
                    prod = scrp.tile([P, 867], f32, tag="prod")
                    nc.vector.scalar_tensor_tensor(
                        out=prod[:, 0:w],
                        in0=accs[h][:, lo:hi],
                        scalar=1.0,
                        in1=dist[:, lo:hi],
                        op0=mybir.AluOpType.mult,
                        op1=mybir.AluOpType.mult,
                        accum_out=parts[:, h * NCH + c: h * NCH + c + 1],
                    )
                    nc.vector.tensor_reduce(
                        out=parts[:, NCH * H + h * NCH + c:
                                  NCH * H + h * NCH + c + 1],
                        in_=accs[h][:, lo:hi],
                        axis=mybir.AxisListType.X,
                        op=mybir.AluOpType.add,
                    )
            nc.sync.dma_start(part_t.ap(), parts[:])
    nc.compile()
    return nc


def _build_phase_b():
    import concourse.bacc as bacc
    import concourse.mybir as mybir
    import concourse.tile as tile

    f32 = mybir.dt.float32
    nc = bacc.Bacc("TRN2", target_bir_lowering=False, debug=False,
                   num_devices=N_CORES)
    x_t = nc.dram_tensor("x_s", [ROWS, C], f32, kind="ExternalInput")
    u_t = nc.dram_tensor("u_s", [ROWS, C], f32, kind="ExternalInput")
    p_t = nc.dram_tensor("pfull", [P, C], f32, kind="ExternalInput")
    inv_t = nc.dram_tensor("invfull", [P, C], f32, kind="ExternalInput")
    out_t = nc.dram_tensor("out_s", [ROWS, C], f32, kind="ExternalOutput")

    n_tiles = (ROWS + P - 1) // P  # 19 (18 full + one 4-row tile)

    with tile.TileContext(nc) as tc:
        with (
            tc.tile_pool(name="const", bufs=1) as constp,
            tc.tile_pool(name="xin", bufs=3) as xp,
            tc.tile_pool(name="uin", bufs=3) as up,
            tc.tile_pool(name="work", bufs=6) as wp,
        ):
            pf = constp.tile([P, C], f32)
            nc.sync.dma_start(pf[:], p_t.ap())
            invf = constp.tile([P, C], f32)
            nc.sync.dma_start(invf[:], inv_t.ap())

            for t in range(n_tiles):
                r0 = t * P
                r1 = min(ROWS, r0 + P)
                rows = r1 - r0
                x_ = xp.tile([P, C], f32, tag="x")
                nc.sync.dma_start(x_[0:rows, :], x_t.ap()[r0:r1, :])
                u_ = up.tile([P, C], f32, tag="u")
                nc.sync.dma_start(u_[0:rows, :], u_t.ap()[r0:r1, :])

                m_ = wp.tile([P, C], f32, tag="m")
                nc.vector.tensor_tensor(m_[0:rows, :], u_[0:rows, :],
                                        pf[0:rows, :],
                                        op=mybir.AluOpType.is_ge)
                s_ = wp.tile([P, C], f32, tag="s")
                nc.vector.tensor_tensor(s_[0:rows, :], x_[0:rows, :],
                                        invf[0:rows, :],
                                        op=mybir.AluOpType.mult)
                o_ = wp.tile([P, C], f32, tag="o")
                nc.vector.tensor_tensor(o_[0:rows, :], m_[0:rows, :],
                                        s_[0:rows, :],
                                        op=mybir.AluOpType.mult)
                nc.sync.dma_start(out_t.ap()[r0:r1, :], o_[0:rows, :])
    nc.compile()
    return nc


def _get_state():
    if "nc_a" not in _STATE:
        dist_flat, u_perm = _build_consts()
        _STATE["dist_flat"] = dist_flat
        _STATE["u_perm"] = u_perm
        _STATE["nc_a"] = _build_phase_a()
        _STATE["nc_b"] = _build_phase_b()
    return _STATE


def _run_phases(x, attn, trace=False, trace_kwargs=None):
    from concourse.bass_utils import run_bass_kernel_spmd

    st = _get_state()
    dist_flat = st["dist_flat"]
    u_perm = st["u_perm"]

    x = np.ascontiguousarray(x, dtype=np.float32)
    attn = np.ascontiguousarray(attn, dtype=np.float32)

    tkw = dict(trace_kwargs or {})

    # ---- phase A: per-head num/den partial reductions ----
    in_maps_a = []
    for i in range(N_CORES):
        attn_s = attn[i * B_LOC:(i + 1) * B_LOC].reshape(B_LOC * H, NN)
        in_maps_a.append({"attn_s": attn_s, "dist": dist_flat})
    res_a = run_bass_kernel_spmd(st["nc_a"], in_maps_a,
                                 core_ids=list(range(N_CORES)),
                                 trace=trace, **tkw)

    # ---- host: combine partials in float64, compute p ----
    num = np.zeros(H, dtype=np.float64)
    den = np.zeros(H, dtype=np.float64)
    for i in range(N_CORES):
        parts = res_a.results[i]["partials"].astype(np.float64)
        for h in range(H):
            num[h] += parts[:, h * NCH:(h + 1) * NCH].sum()
            den[h] += parts[:, NCH * H + h * NCH:
                            NCH * H + (h + 1) * NCH].sum()
    # straggler element (i=j=576): dist==0 there, denominator only
    den += attn[:, :, N - 1, N - 1].astype(np.float64).sum(axis=0)

    avg = num / den
    p64 = 0.5 / np.exp(5.0 * avg / SIDE)
    p = p64.astype(np.float32)  # [H]
    om = (np.float32(1.0) - p).astype(np.float32)
    inv = (np.float32(1.0) / om).astype(np.float32)

    p_vec = np.repeat(p, Ch).astype(np.float32)  # [C]
    inv_vec = np.repeat(inv, Ch).astype(np.float32)
    pfull = np.ascontiguousarray(np.broadcast_to(p_vec, (P, C)))
    invfull = np.ascontiguousarray(np.broadcast_to(inv_vec, (P, C)))

    # ---- phase B: dropout ----
    in_maps_b = []
    for i in range(N_CORES):
        x_s = x[i * B_LOC:(i + 1) * B_LOC].reshape(ROWS, C)
        u_s = u_perm[i * B_LOC:(i + 1) * B_LOC].reshape(ROWS, C)
        in_maps_b.append({"x_s": x_s, "u_s": u_s,
                          "pfull": pfull, "invfull": invfull})
    res_b = run_bass_kernel_spmd(st["nc_b"], in_maps_b,
                                 core_ids=list(range(N_CORES)),
                                 trace=trace, **tkw)

    out = np.empty((B, N, C), dtype=np.float32)
    for i in range(N_CORES):
        out[i * B_LOC:(i + 1) * B_LOC] = (
            res_b.results[i]["out_s"].reshape(B_LOC, N, C))
    return out, res_a, res_b, dict(p=p, num=num, den=den)


def kernel(x, attn):
    out, _, _, _ = _run_phases(x, attn, trace=False)
    return out


# revision 9
# speedup vs baseline: 1.2823x; 1.2823x over previous
"""Trainium2 Bass kernel for nn_CustomDropout: per-head attention-distance
dropout.

Reference semantics:
  p[h]   = 0.5 / exp(5 * avgdist[h] / sqrt(N)),
  avgdist[h] = (sum_{b,i,j} attn[b,h,i,j] * dist[i,j]) / (sum attn[b,h,:,:])
  out[b,n,h*Ch+ch] = where(u[h,b,n,ch] >= p[h], x[b,n,c]/(1-p[h]), 0)
  with u = jax.random.uniform(key(42), [H,B,N,Ch]) (input-independent).

Plan (8 NeuronCores, data-parallel over B, 4 batches/core):
  Phase A (bass kernel): load the 4 local batch slices of attn per head as
    [128, 2601] tiles (HWDGE), fold them with adds split across VectorE and
    GpSimd, fused multiply+reduce against dist on VectorE for the numerator
    (chunked for fp32 accuracy), denominator via exact fp32 ones-matmul on
    TensorE -> per-partition partial sums.
  Host: combine partials over cores/partitions/chunks in float64, add the
    straggler element (577^2 = 128*2601 + 1; dist there is 0), compute p.
  Phase B (bass kernel): elementwise dropout: mask = (u >= p_c),
    out = mask * (x * 1/(1-p_c)); x*(1/(1-p)) on GpSimd, rest on VectorE,
    row-tiles interleaved six-at-a-time into [128, 4608] tiles.
"""

import numpy as np

B, H, N, C = 32, 12, 577, 768
Ch = C // H  # 64
N_CORES = 8
B_LOC = B // N_CORES  # 4
NN = N * N  # 332929
P = 128
F = 2601  # NN // 128
NN_MAIN = P * F  # 332928; one straggler element (i=j=576) handled on host
SIDE = N ** 0.5
ROWS = B_LOC * N  # 2308 rows per core in phase B
CHUNKS = [(0, 867), (867, 1734), (1734, 2601)]
NCH = len(CHUNKS)
DEN_CHUNKS = [(0, 512), (512, 1024), (1024, 1536), (1536, 2048),
              (2048, 2560), (2560, 2601)]
NDC = 4  # den partial columns per head (psum [1,512] reduced in 4 pieces)
KGRP = 6  # phase B: row-tiles interleaved per DMA/op group

_STATE = {}


def _build_consts():
    """dist matrix and uniform draws, computed with the exact same jax ops
    (and backend) as the reference so they match bitwise."""
    import jax
    import jax.numpy as jnp

    idx = jnp.arange(N, dtype=jnp.float32)
    d = idx[None, :] - idx[:, None]
    dist = jnp.sqrt((d % SIDE) ** 2 + jnp.floor_divide(d, SIDE) ** 2)
    dist = np.asarray(dist, dtype=np.float32)
    assert dist.shape == (N, N)

    u = jax.random.uniform(
        jax.random.key(42), (H, B, N, Ch), dtype=jnp.float32
    )
    u = np.asarray(u)
    # u_perm[b, n, h*Ch+ch] = u[h, b, n, ch]
    u_perm = np.ascontiguousarray(u.transpose(1, 2, 0, 3)).reshape(B, N, C)

    dist_flat = np.ascontiguousarray(dist.reshape(-1)[:NN_MAIN]).reshape(P, F)
    # the dropped element is (i=576, j=576): d=0 -> dist 0, so it only
    # affects the denominator (added on host)
    assert dist.reshape(-1)[NN_MAIN] == 0.0
    return dist_flat, u_perm


def _build_phase_a():
    import concourse.bacc as bacc
    import concourse.mybir as mybir
    import concourse.tile as tile

    f32 = mybir.dt.float32
    nc = bacc.Bacc("TRN2", target_bir_lowering=False, debug=False,
                   num_devices=N_CORES)
    attn_t = nc.dram_tensor("attn_s", [B_LOC * H, NN], f32,
                            kind="ExternalInput")
    dist_t = nc.dram_tensor("dist", [P, F], f32, kind="ExternalInput")
    # cols [0, 3H): numerator chunks (all partitions)
    # cols [3H, 3H + 4H): denominator pieces (partition 0 only)
    part_t = nc.dram_tensor("partials", [P, NCH * H + NDC * H], f32,
                            kind="ExternalOutput")

    with tile.TileContext(nc) as tc:
        with (
            tc.tile_pool(name="const", bufs=1) as constp,
            tc.tile_pool(name="bt", bufs=12) as btp,
            tc.tile_pool(name="scr", bufs=3) as scrp,
            tc.tile_pool(name="outp", bufs=1) as outp,
            tc.tile_pool(name="ps", bufs=4, space="PSUM") as psp,
        ):
            dist = constp.tile([P, F], f32)
            nc.sync.dma_start(dist[:], dist_t.ap())
            ones = constp.tile([P, 1], f32)
            nc.gpsimd.memset(ones[:], 1.0)
            parts = outp.tile([P, NCH * H + NDC * H], f32)
            nc.gpsimd.memset(parts[:], 0.0)

            for h in range(H):
                bt = []
                for b in range(B_LOC):
                    t = btp.tile([P, F], f32, tag="bt", name=f"bt{h}_{b}")
                    src = attn_t.ap()[b * H + h, 0:NN_MAIN].rearrange(
                        "(p f) -> p f", p=P)
                    eng = nc.sync if b % 2 == 0 else nc.scalar
                    eng.dma_start(t[:], src)
                    bt.append(t)
                # fold 4 batches: b2+b3 on GpSimd, the rest on VectorE
                nc.gpsimd.tensor_tensor(bt[2][:], bt[2][:], bt[3][:],
                                        op=mybir.AluOpType.add)
                nc.vector.tensor_tensor(bt[0][:], bt[0][:], bt[1][:],
                                        op=mybir.AluOpType.add)
                acc = bt[0]
                nc.vector.tensor_tensor(acc[:], acc[:], bt[2][:],
                                        op=mybir.AluOpType.add)
                # numerator: fused (acc * dist) + per-partition reduce
                for c, (lo, hi) in enumerate(CHUNKS):
                    w = hi - lo
                    prod = scrp.tile([P, 867], f32, tag="prod")
                    nc.vector.scalar_tensor_tensor(
                        out=prod[:, 0:w],
                        in0=acc[:, lo:hi],
                        scalar=1.0,
                        in1=dist[:, lo:hi],
                        op0=mybir.AluOpType.mult,
                        op1=mybir.AluOpType.mult,
                        accum_out=parts[:, h * NCH + c: h * NCH + c + 1],
                    )
                # denominator: exact fp32 ones-matmul partition reduction
                ps = psp.tile([1, 512], f32, tag="ps", name=f"ps{h}")
                for c, (lo, hi) in enumerate(DEN_CHUNKS):
                    nc.tensor.matmul(ps[0:1, 0:hi - lo], ones[:],
                                     acc[:, lo:hi],
                                     start=(c == 0),
                                     stop=(c == len(DEN_CHUNKS) - 1))
                for j in range(NDC):
                    col = NCH * H + h * NDC + j
                    nc.vector.tensor_reduce(
                        out=parts[0:1, col:col + 1],
                        in_=ps[0:1, j * 128:(j + 1) * 128],
                        axis=mybir.AxisListType.X,
                        op=mybir.AluOpType.add,
                    )
            nc.sync.dma_start(part_t.ap(), parts[:])
    nc.compile()
    return nc


def _build_phase_b():
    import concourse.bacc as bacc
    import concourse.mybir as mybir
    import concourse.tile as tile

    f32 = mybir.dt.float32
    nc = bacc.Bacc("TRN2", target_bir_lowering=False, debug=False,
                   num_devices=N_CORES)
    x_t = nc.dram_tensor("x_s", [ROWS, C], f32, kind="ExternalInput")
    u_t = nc.dram_tensor("u_s", [ROWS, C], f32, kind="ExternalInput")
    p_t = nc.dram_tensor("pfull", [P, C], f32, kind="ExternalInput")
    inv_t = nc.dram_tensor("invfull", [P, C], f32, kind="ExternalInput")
    out_t = nc.dram_tensor("out_s", [ROWS, C], f32, kind="ExternalOutput")

    n_grp = ROWS // (P * KGRP)  # 3 groups of 6 interleaved row-tiles
    tail0 = n_grp * P * KGRP  # 2304
    W = KGRP * C  # 4608

    with tile.TileContext(nc) as tc:
        with (
            tc.tile_pool(name="const", bufs=1) as constp,
            tc.tile_pool(name="xin", bufs=2) as xp,
            tc.tile_pool(name="uin", bufs=2) as up,
            tc.tile_pool(name="work", bufs=2) as wp,
            tc.tile_pool(name="tails", bufs=1) as tp,
        ):
            pf = constp.tile([P, C], f32)
            nc.sync.dma_start(pf[:], p_t.ap())
            invf = constp.tile([P, C], f32)
            nc.scalar.dma_start(invf[:], inv_t.ap())
            pfb = pf[:, None, :].broadcast_to([P, KGRP, C])
            invfb = invf[:, None, :].broadcast_to([P, KGRP, C])

            for g in range(n_grp):
                r0 = g * P * KGRP
                r1 = r0 + P * KGRP
                # tile[p, k, c] = x[r0 + k*128 + p, c]
                xs_ap = x_t.ap()[r0:r1, :].rearrange("(k p) c -> p k c", p=P)
                us_ap = u_t.ap()[r0:r1, :].rearrange("(k p) c -> p k c", p=P)
                x_ = xp.tile([P, KGRP, C], f32, tag="x")
                nc.sync.dma_start(x_[:], xs_ap)
                u_ = up.tile([P, KGRP, C], f32, tag="u")
                nc.scalar.dma_start(u_[:], us_ap)

                m_ = wp.tile([P, KGRP, C], f32, tag="m")
                nc.vector.tensor_tensor(m_[:], u_[:], pfb,
                                        op=mybir.AluOpType.is_ge)
                s_ = wp.tile([P, KGRP, C], f32, tag="s")
                nc.gpsimd.tensor_tensor(s_[:], x_[:], invfb,
                                        op=mybir.AluOpType.mult)
                nc.vector.tensor_tensor(m_[:], m_[:], s_[:],
                                        op=mybir.AluOpType.mult)
                nc.sync.dma_start(
                    out_t.ap()[r0:r1, :].rearrange("(k p) c -> p k c", p=P),
                    m_[:])

            # tail rows (2304..2307)
            rows = ROWS - tail0
            x_ = tp.tile([P, C], f32, tag="xt", name="x_tail")
            nc.sync.dma_start(x_[0:rows, :], x_t.ap()[tail0:ROWS, :])
            u_ = tp.tile([P, C], f32, tag="ut", name="u_tail")
            nc.scalar.dma_start(u_[0:rows, :], u_t.ap()[tail0:ROWS, :])
            m_ = tp.tile([P, C], f32, tag="mt", name="m_tail")
            nc.vector.tensor_tensor(m_[0:rows, :], u_[0:rows, :],
                                    pf[0:rows, :], op=mybir.AluOpType.is_ge)
            s_ = tp.tile([P, C], f32, tag="st", name="s_tail")
            nc.vector.tensor_tensor(s_[0:rows, :], x_[0:rows, :],
                                    invf[0:rows, :], op=mybir.AluOpType.mult)
            nc.vector.tensor_tensor(m_[0:rows, :], m_[0:rows, :],
                                    s_[0:rows, :], op=mybir.AluOpType.mult)
            nc.sync.dma_start(out_t.ap()[tail0:ROWS, :], m_[0:rows, :])
    nc.compile()
    return nc


def _get_state():
    if "nc_a" not in _STATE:
        dist_flat, u_perm = _build_consts()
        _STATE["dist_flat"] = dist_flat
        _STATE["u_perm"] = u_perm
        _STATE["nc_a"] = _build_phase_a()
        _STATE["nc_b"] = _build_phase_b()
    return _STATE


def _run_phases(x, attn, trace=False, trace_kwargs=None):
    from concourse.bass_utils import run_bass_kernel_spmd

    st = _get_state()
    dist_flat = st["dist_flat"]
    u_perm = st["u_perm"]

    x = np.ascontiguousarray(x, dtype=np.float32)
    attn = np.ascontiguousarray(attn, dtype=np.float32)

    tkw = dict(trace_kwargs or {})

    # ---- phase A: per-head num/den partial reductions ----
    in_maps_a = []
    for i in range(N_CORES):
        attn_s = attn[i * B_LOC:(i + 1) * B_LOC].reshape(B_LOC * H, NN)
        in_maps_a.append({"attn_s": attn_s, "dist": dist_flat})
    res_a = run_bass_kernel_spmd(st["nc_a"], in_maps_a,
                                 core_ids=list(range(N_CORES)),
                                 trace=trace, **tkw)

    # ---- host: combine partials in float64, compute p ----
    num = np.zeros(H, dtype=np.float64)
    den = np.zeros(H, dtype=np.float64)
    for i in range(N_CORES):
        parts = res_a.results[i]["partials"].astype(np.float64)
        for h in range(H):
            num[h] += parts[:, h * NCH:(h + 1) * NCH].sum()
            den[h] += parts[0, NCH * H + h * NDC:
                            NCH * H + (h + 1) * NDC].sum()
    # straggler element (i=j=576): dist==0 there, denominator only
    den += attn[:, :, N - 1, N - 1].astype(np.float64).sum(axis=0)

    avg = num / den
    p64 = 0.5 / np.exp(5.0 * avg / SIDE)
    p = p64.astype(np.float32)  # [H]
    om = (np.float32(1.0) - p).astype(np.float32)
    inv = (np.float32(1.0) / om).astype(np.float32)

    p_vec = np.repeat(p, Ch).astype(np.float32)  # [C]
    inv_vec = np.repeat(inv, Ch).astype(np.float32)
    pfull = np.ascontiguousarray(np.broadcast_to(p_vec, (P, C)))
    invfull = np.ascontiguousarray(np.broadcast_to(inv_vec, (P, C)))

    # ---- phase B: dropout ----
    in_maps_b = []
    for i in range(N_CORES):
        x_s = x[i * B_LOC:(i + 1) * B_LOC].reshape(ROWS, C)
        u_s = u_perm[i * B_LOC:(i + 1) * B_LOC].reshape(ROWS, C)
        in_maps_b.append({"x_s": x_s, "u_s": u_s,
                          "pfull": pfull, "invfull": invfull})
    res_b = run_bass_kernel_spmd(st["nc_b"], in_maps_b,
                                 core_ids=list(range(N_CORES)),
                                 trace=trace, **tkw)

    out = np.empty((B, N, C), dtype=np.float32)
    for i in range(N_CORES):
        out[i * B_LOC:(i + 1) * B_LOC] = (
            res_b.results[i]["out_s"].reshape(B_LOC, N, C))
    return out, res_a, res_b, dict(p=p, num=num, den=den)


def kernel(x, attn):
    out, _, _, _ = _run_phases(x, attn, trace=False)
    return out


# revision 10
# speedup vs baseline: 1.3895x; 1.0836x over previous
"""Trainium2 Bass kernel for nn_CustomDropout: per-head attention-distance
dropout.

Reference semantics:
  p[h]   = 0.5 / exp(5 * avgdist[h] / sqrt(N)),
  avgdist[h] = (sum_{b,i,j} attn[b,h,i,j] * dist[i,j]) / (sum attn[b,h,:,:])
  out[b,n,h*Ch+ch] = where(u[h,b,n,ch] >= p[h], x[b,n,c]/(1-p[h]), 0)
  with u = jax.random.uniform(key(42), [H,B,N,Ch]) (input-independent).

Plan (8 NeuronCores, data-parallel over B, 4 batches/core):
  Phase A (bass kernel): load the 4 local batch slices of attn per head as
    [128, 2601] tiles (HWDGE), fold them with adds split across VectorE and
    GpSimd, fused multiply+reduce against dist on VectorE for the numerator
    (chunked for fp32 accuracy), denominator via exact fp32 ones-matmul on
    TensorE -> per-partition partial sums.
  Host: combine partials over cores/partitions/chunks in float64, add the
    straggler element (577^2 = 128*2601 + 1; dist there is 0), compute p.
  Phase B (bass kernel): elementwise dropout: mask = (u >= p_c),
    out = mask * (x * 1/(1-p_c)); x*(1/(1-p)) on GpSimd, rest on VectorE,
    row-tiles interleaved six-at-a-time into [128, 4608] tiles.
"""

import numpy as np

B, H, N, C = 32, 12, 577, 768
Ch = C // H  # 64
N_CORES = 8
B_LOC = B // N_CORES  # 4
NN = N * N  # 332929
P = 128
F = 2601  # NN // 128
NN_MAIN = P * F  # 332928; one straggler element (i=j=576) handled on host
SIDE = N ** 0.5
ROWS = B_LOC * N  # 2308 rows per core in phase B
CHUNKS = [(0, 867), (867, 1734), (1734, 2601)]
NCH = len(CHUNKS)
DEN_CHUNKS = [(0, 512), (512, 1024), (1024, 1536), (1536, 2048),
              (2048, 2560), (2560, 2601)]
NDC = 4  # den partial columns per head (psum [1,512] reduced in 4 pieces)
KGRP = 6  # phase B: row-tiles interleaved per DMA/op group

_STATE = {}


def _build_consts():
    """dist matrix and uniform draws, computed with the exact same jax ops
    (and backend) as the reference so they match bitwise."""
    import jax
    import jax.numpy as jnp

    idx = jnp.arange(N, dtype=jnp.float32)
    d = idx[None, :] - idx[:, None]
    dist = jnp.sqrt((d % SIDE) ** 2 + jnp.floor_divide(d, SIDE) ** 2)
    dist = np.asarray(dist, dtype=np.float32)
    assert dist.shape == (N, N)

    u = jax.random.uniform(
        jax.random.key(42), (H, B, N, Ch), dtype=jnp.float32
    )
    u = np.asarray(u)
    # u_perm[b, n, h*Ch+ch] = u[h, b, n, ch]
    u_perm = np.ascontiguousarray(u.transpose(1, 2, 0, 3)).reshape(B, N, C)

    dist_flat = np.ascontiguousarray(dist.reshape(-1)[:NN_MAIN]).reshape(P, F)
    # the dropped element is (i=576, j=576): d=0 -> dist 0, so it only
    # affects the denominator (added on host)
    assert dist.reshape(-1)[NN_MAIN] == 0.0
    return dist_flat, u_perm


def _build_phase_a():
    import concourse.bacc as bacc
    import concourse.mybir as mybir
    import concourse.tile as tile

    f32 = mybir.dt.float32
    nc = bacc.Bacc("TRN2", target_bir_lowering=False, debug=False,
                   num_devices=N_CORES)
    attn_t = nc.dram_tensor("attn_s", [B_LOC * H, NN], f32,
                            kind="ExternalInput")
    dist_t = nc.dram_tensor("dist", [P, F], f32, kind="ExternalInput")
    # cols [0, 3H): numerator chunks (all partitions)
    # cols [3H, 3H + 4H): denominator pieces (partition 0 only)
    part_t = nc.dram_tensor("partials", [P, NCH * H + NDC * H], f32,
                            kind="ExternalOutput")

    with tile.TileContext(nc) as tc:
        with (
            tc.tile_pool(name="const", bufs=1) as constp,
            tc.tile_pool(name="bt", bufs=12) as btp,
            tc.tile_pool(name="scr", bufs=3) as scrp,
            tc.tile_pool(name="outp", bufs=1) as outp,
            tc.tile_pool(name="ps", bufs=4, space="PSUM") as psp,
        ):
            dist = constp.tile([P, F], f32)
            nc.sync.dma_start(dist[:], dist_t.ap())
            ones = constp.tile([P, 1], f32)
            nc.gpsimd.memset(ones[:], 1.0)
            parts = outp.tile([P, NCH * H + NDC * H], f32)
            nc.gpsimd.memset(parts[:], 0.0)

            for h in range(H):
                bt = []
                for b in range(B_LOC):
                    t = btp.tile([P, F], f32, tag="bt", name=f"bt{h}_{b}")
                    src = attn_t.ap()[b * H + h, 0:NN_MAIN].rearrange(
                        "(p f) -> p f", p=P)
                    eng = nc.sync if b % 2 == 0 else nc.scalar
                    eng.dma_start(t[:], src)
                    bt.append(t)
                # fold 4 batches: b2+b3 on GpSimd, the rest on VectorE
                nc.gpsimd.tensor_tensor(bt[2][:], bt[2][:], bt[3][:],
                                        op=mybir.AluOpType.add)
                nc.vector.tensor_tensor(bt[0][:], bt[0][:], bt[1][:],
                                        op=mybir.AluOpType.add)
                acc = bt[0]
                nc.vector.tensor_tensor(acc[:], acc[:], bt[2][:],
                                        op=mybir.AluOpType.add)
                # numerator: fused (acc * dist) + per-partition reduce
                for c, (lo, hi) in enumerate(CHUNKS):
                    w = hi - lo
                    prod = scrp.tile([P, 867], f32, tag="prod")
                    nc.vector.scalar_tensor_tensor(
                        out=prod[:, 0:w],
                        in0=acc[:, lo:hi],
                        scalar=1.0,
                        in1=dist[:, lo:hi],
                        op0=mybir.AluOpType.mult,
                        op1=mybir.AluOpType.mult,
                        accum_out=parts[:, h * NCH + c: h * NCH + c + 1],
                    )
                # denominator: exact fp32 ones-matmul partition reduction
                ps = psp.tile([1, 512], f32, tag="ps", name=f"ps{h}")
                for c, (lo, hi) in enumerate(DEN_CHUNKS):
                    nc.tensor.matmul(ps[0:1, 0:hi - lo], ones[:],
                                     acc[:, lo:hi],
                                     start=(c == 0),
                                     stop=(c == len(DEN_CHUNKS) - 1))
                for j in range(NDC):
                    col = NCH * H + h * NDC + j
                    nc.vector.tensor_reduce(
                        out=parts[0:1, col:col + 1],
                        in_=ps[0:1, j * 128:(j + 1) * 128],
                        axis=mybir.AxisListType.X,
                        op=mybir.AluOpType.add,
                    )
            nc.sync.dma_start(part_t.ap(), parts[:])
    nc.compile()
    return nc


def _build_phase_b():
    import concourse.bacc as bacc
    import concourse.mybir as mybir
    import concourse.tile as tile

    f32 = mybir.dt.float32
    nc = bacc.Bacc("TRN2", target_bir_lowering=False, debug=False,
                   num_devices=N_CORES)
    x_t = nc.dram_tensor("x_s", [ROWS, C], f32, kind="ExternalInput")
    u_t = nc.dram_tensor("u_s", [ROWS, C], f32, kind="ExternalInput")
    p_t = nc.dram_tensor("pfull", [P, C], f32, kind="ExternalInput")
    inv_t = nc.dram_tensor("invfull", [P, C], f32, kind="ExternalInput")
    out_t = nc.dram_tensor("out_s", [ROWS, C], f32, kind="ExternalOutput")

    n_grp = ROWS // (P * KGRP)  # 3 groups of 6 interleaved row-tiles
    tail0 = n_grp * P * KGRP  # 2304
    W = KGRP * C  # 4608

    with tile.TileContext(nc) as tc:
        with (
            tc.tile_pool(name="const", bufs=1) as constp,
            tc.tile_pool(name="xin", bufs=2) as xp,
            tc.tile_pool(name="uin", bufs=2) as up,
            tc.tile_pool(name="work", bufs=2) as wp,
            tc.tile_pool(name="tails", bufs=1) as tp,
        ):
            pf = constp.tile([P, C], f32)
            nc.sync.dma_start(pf[:], p_t.ap())
            invf = constp.tile([P, C], f32)
            nc.scalar.dma_start(invf[:], inv_t.ap())
            pfb = pf[:, None, :].broadcast_to([P, KGRP, C])
            invfb = invf[:, None, :].broadcast_to([P, KGRP, C])

            for g in range(n_grp):
                r0 = g * P * KGRP
                r1 = r0 + P * KGRP
                # tile[p, k, c] = x[r0 + k*128 + p, c]
                xs_ap = x_t.ap()[r0:r1, :].rearrange("(k p) c -> p k c", p=P)
                us_ap = u_t.ap()[r0:r1, :].rearrange("(k p) c -> p k c", p=P)
                x_ = xp.tile([P, KGRP, C], f32, tag="x")
                nc.sync.dma_start(x_[:], xs_ap)
                u_ = up.tile([P, KGRP, C], f32, tag="u")
                nc.scalar.dma_start(u_[:], us_ap)

                m_ = wp.tile([P, KGRP, C], f32, tag="m")
                nc.vector.tensor_tensor(m_[:], u_[:], pfb,
                                        op=mybir.AluOpType.is_ge)
                s_ = wp.tile([P, KGRP, C], f32, tag="s")
                nc.vector.tensor_tensor(s_[:], x_[:], invfb,
                                        op=mybir.AluOpType.mult)
                nc.vector.tensor_tensor(m_[:], m_[:], s_[:],
                                        op=mybir.AluOpType.mult)
                nc.sync.dma_start(
                    out_t.ap()[r0:r1, :].rearrange("(k p) c -> p k c", p=P),
                    m_[:])

            # tail rows (2304..2307)
            rows = ROWS - tail0
            x_ = tp.tile([P, C], f32, tag="xt", name="x_tail")
            nc.sync.dma_start(x_[0:rows, :], x_t.ap()[tail0:ROWS, :])
            u_ = tp.tile([P, C], f32, tag="ut", name="u_tail")
            nc.scalar.dma_start(u_[0:rows, :], u_t.ap()[tail0:ROWS, :])
            m_ = tp.tile([P, C], f32, tag="mt", name="m_tail")
            nc.vector.tensor_tensor(m_[0:rows, :], u_[0:rows, :],
                                    pf[0:rows, :], op=mybir.AluOpType.is_ge)
            s_ = tp.tile([P, C], f32, tag="st", name="s_tail")
            nc.vector.tensor_tensor(s_[0:rows, :], x_[0:rows, :],
                                    invf[0:rows, :], op=mybir.AluOpType.mult)
            nc.vector.tensor_tensor(m_[0:rows, :], m_[0:rows, :],
                                    s_[0:rows, :], op=mybir.AluOpType.mult)
            nc.sync.dma_start(out_t.ap()[tail0:ROWS, :], m_[0:rows, :])
    nc.compile()
    return nc


def _get_state():
    if "nc_a" not in _STATE:
        dist_flat, u_perm = _build_consts()
        _STATE["dist_flat"] = dist_flat
        _STATE["u_perm"] = u_perm
        _STATE["nc_a"] = _build_phase_a()
        _STATE["nc_b"] = _build_phase_b()
    return _STATE


def _run_phases(x, attn, trace=False, trace_kwargs=None):
    from concourse.bass_utils import run_bass_kernel_spmd

    st = _get_state()
    dist_flat = st["dist_flat"]
    u_perm = st["u_perm"]

    x = np.ascontiguousarray(x, dtype=np.float32)
    attn = np.ascontiguousarray(attn, dtype=np.float32)

    tkw = dict(trace_kwargs or {})

    # ---- phase A: per-head num/den partial reductions ----
    in_maps_a = []
    for i in range(N_CORES):
        attn_s = attn[i * B_LOC:(i + 1) * B_LOC].reshape(B_LOC * H, NN)
        in_maps_a.append({"attn_s": attn_s, "dist": dist_flat})
    res_a = run_bass_kernel_spmd(st["nc_a"], in_maps_a,
                                 core_ids=list(range(N_CORES)),
                                 trace=trace, **tkw)

    # ---- host: combine partials in float64, compute p ----
    num = np.zeros(H, dtype=np.float64)
    den = np.zeros(H, dtype=np.float64)
    for i in range(N_CORES):
        parts = res_a.results[i]["partials"].astype(np.float64)
        for h in range(H):
            num[h] += parts[:, h * NCH:(h + 1) * NCH].sum()
            den[h] += parts[0, NCH * H + h * NDC:
                            NCH * H + (h + 1) * NDC].sum()
    # straggler element (i=j=576): dist==0 there, denominator only
    den += attn[:, :, N - 1, N - 1].astype(np.float64).sum(axis=0)

    avg = num / den
    p64 = 0.5 / np.exp(5.0 * avg / SIDE)
    p = p64.astype(np.float32)  # [H]
    om = (np.float32(1.0) - p).astype(np.float32)
    inv = (np.float32(1.0) / om).astype(np.float32)

    p_vec = np.repeat(p, Ch).astype(np.float32)  # [C]
    inv_vec = np.repeat(inv, Ch).astype(np.float32)
    pfull = np.ascontiguousarray(np.broadcast_to(p_vec, (P, C)))
    invfull = np.ascontiguousarray(np.broadcast_to(inv_vec, (P, C)))

    # ---- phase B: dropout ----
    in_maps_b = []
    for i in range(N_CORES):
        x_s = x[i * B_LOC:(i + 1) * B_LOC].reshape(ROWS, C)
        u_s = u_perm[i * B_LOC:(i + 1) * B_LOC].reshape(ROWS, C)
        in_maps_b.append({"x_s": x_s, "u_s": u_s,
                          "pfull": pfull, "invfull": invfull})
    res_b = run_bass_kernel_spmd(st["nc_b"], in_maps_b,
                                 core_ids=list(range(N_CORES)),
                                 trace=trace, **tkw)

    out = np.empty((B, N, C), dtype=np.float32)
    for i in range(N_CORES):
        out[i * B_LOC:(i + 1) * B_LOC] = (
            res_b.results[i]["out_s"].reshape(B_LOC, N, C))
    return out, res_a, res_b, dict(p=p, num=num, den=den)


def kernel(x, attn):
    out, _, _, _ = _run_phases(x, attn, trace=False)
    return out


# revision 11
# speedup vs baseline: 1.4856x; 1.0691x over previous
"""Trainium2 Bass kernel for nn_CustomDropout: per-head attention-distance
dropout.

Reference semantics:
  p[h]   = 0.5 / exp(5 * avgdist[h] / sqrt(N)),
  avgdist[h] = (sum_{b,i,j} attn[b,h,i,j] * dist[i,j]) / (sum attn[b,h,:,:])
  out[b,n,h*Ch+ch] = where(u[h,b,n,ch] >= p[h], x[b,n,c]/(1-p[h]), 0)
  with u = jax.random.uniform(key(42), [H,B,N,Ch]) (input-independent).

Plan (8 NeuronCores, data-parallel over B, 4 batches/core):
  Phase A (bass kernel): load the 4 local batch slices of attn per head as
    [128, 2601] tiles (HWDGE), fold them with adds split across VectorE and
    GpSimd, fused multiply+reduce against dist on VectorE for the numerator
    (chunked for fp32 accuracy), denominator via exact fp32 ones-matmul on
    TensorE -> per-partition partial sums.
  Host: combine partials over cores/partitions/chunks in float64, add the
    straggler element (577^2 = 128*2601 + 1; dist there is 0), compute p.
  Phase B (bass kernel): elementwise dropout: mask = (u >= p_c),
    out = mask * (x * 1/(1-p_c)); x*(1/(1-p)) on GpSimd, rest on VectorE,
    row-tiles interleaved six-at-a-time into [128, 4608] tiles.
"""

import numpy as np

B, H, N, C = 32, 12, 577, 768
Ch = C // H  # 64
N_CORES = 8
B_LOC = B // N_CORES  # 4
NN = N * N  # 332929
P = 128
F = 2601  # NN // 128
NN_MAIN = P * F  # 332928; one straggler element (i=j=576) handled on host
SIDE = N ** 0.5
ROWS = B_LOC * N  # 2308 rows per core in phase B
CHUNKS = [(0, 867), (867, 1734), (1734, 2601)]
NCH = len(CHUNKS)
DEN_CHUNKS = [(0, 512), (512, 1024), (1024, 1536), (1536, 2048),
              (2048, 2560), (2560, 2601)]
NDC = 4  # den partial columns per head (psum [1,512] reduced in 4 pieces)
KGRP = 6  # phase B: row-tiles interleaved per DMA/op group

_STATE = {}


def _build_consts():
    """dist matrix and uniform draws, computed with the exact same jax ops
    (and backend) as the reference so they match bitwise."""
    import jax
    import jax.numpy as jnp

    idx = jnp.arange(N, dtype=jnp.float32)
    d = idx[None, :] - idx[:, None]
    dist = jnp.sqrt((d % SIDE) ** 2 + jnp.floor_divide(d, SIDE) ** 2)
    dist = np.asarray(dist, dtype=np.float32)
    assert dist.shape == (N, N)

    u = jax.random.uniform(
        jax.random.key(42), (H, B, N, Ch), dtype=jnp.float32
    )
    u = np.asarray(u)
    # u_perm[b, n, h*Ch+ch] = u[h, b, n, ch]
    u_perm = np.ascontiguousarray(u.transpose(1, 2, 0, 3)).reshape(B, N, C)

    dist_flat = np.ascontiguousarray(dist.reshape(-1)[:NN_MAIN]).reshape(P, F)
    # the dropped element is (i=576, j=576): d=0 -> dist 0, so it only
    # affects the denominator (added on host)
    assert dist.reshape(-1)[NN_MAIN] == 0.0
    return dist_flat, u_perm


def _build_phase_a():
    import concourse.bacc as bacc
    import concourse.mybir as mybir
    import concourse.tile as tile

    f32 = mybir.dt.float32
    nc = bacc.Bacc("TRN2", target_bir_lowering=False, debug=False,
                   num_devices=N_CORES)
    attn_t = nc.dram_tensor("attn_s", [B_LOC * H, NN], f32,
                            kind="ExternalInput")
    dist_t = nc.dram_tensor("dist", [P, F], f32, kind="ExternalInput")
    # cols [0, 3H): numerator chunks (all partitions)
    # cols [3H, 3H + 4H): denominator pieces (partition 0 only)
    part_t = nc.dram_tensor("partials", [P, NCH * H + NDC * H], f32,
                            kind="ExternalOutput")

    with tile.TileContext(nc) as tc:
        with (
            tc.tile_pool(name="const", bufs=1) as constp,
            tc.tile_pool(name="bt", bufs=12) as btp,
            tc.tile_pool(name="scr", bufs=3) as scrp,
            tc.tile_pool(name="outp", bufs=1) as outp,
            tc.tile_pool(name="ps", bufs=4, space="PSUM") as psp,
        ):
            dist = constp.tile([P, F], f32)
            nc.sync.dma_start(dist[:], dist_t.ap())
            ones = constp.tile([P, 1], f32)
            nc.gpsimd.memset(ones[:], 1.0)
            parts = outp.tile([P, NCH * H + NDC * H], f32)
            nc.gpsimd.memset(parts[:], 0.0)

            for h in range(H):
                bt = []
                for b in range(B_LOC):
                    t = btp.tile([P, F], f32, tag="bt", name=f"bt{h}_{b}")
                    src = attn_t.ap()[b * H + h, 0:NN_MAIN].rearrange(
                        "(p f) -> p f", p=P)
                    eng = nc.sync if b % 2 == 0 else nc.scalar
                    eng.dma_start(t[:], src)
                    bt.append(t)
                # fold 4 batches on VectorE (GpSimd contends with DVE for
                # the shared SBUF port and slows both down)
                nc.vector.tensor_tensor(bt[2][:], bt[2][:], bt[3][:],
                                        op=mybir.AluOpType.add)
                nc.vector.tensor_tensor(bt[0][:], bt[0][:], bt[1][:],
                                        op=mybir.AluOpType.add)
                acc = bt[0]
                nc.vector.tensor_tensor(acc[:], acc[:], bt[2][:],
                                        op=mybir.AluOpType.add)
                # numerator: fused (acc * dist) + per-partition reduce
                for c, (lo, hi) in enumerate(CHUNKS):
                    w = hi - lo
                    prod = scrp.tile([P, 867], f32, tag="prod")
                    nc.vector.scalar_tensor_tensor(
                        out=prod[:, 0:w],
                        in0=acc[:, lo:hi],
                        scalar=1.0,
                        in1=dist[:, lo:hi],
                        op0=mybir.AluOpType.mult,
                        op1=mybir.AluOpType.mult,
                        accum_out=parts[:, h * NCH + c: h * NCH + c + 1],
                    )
                # denominator: exact fp32 ones-matmul partition reduction
                ps = psp.tile([1, 512], f32, tag="ps", name=f"ps{h}")
                for c, (lo, hi) in enumerate(DEN_CHUNKS):
                    nc.tensor.matmul(ps[0:1, 0:hi - lo], ones[:],
                                     acc[:, lo:hi],
                                     start=(c == 0),
                                     stop=(c == len(DEN_CHUNKS) - 1))
                for j in range(NDC):
                    col = NCH * H + h * NDC + j
                    nc.vector.tensor_reduce(
                        out=parts[0:1, col:col + 1],
                        in_=ps[0:1, j * 128:(j + 1) * 128],
                        axis=mybir.AxisListType.X,
                        op=mybir.AluOpType.add,
                    )
            nc.sync.dma_start(part_t.ap(), parts[:])
    nc.compile()
    return nc


def _build_phase_b():
    import concourse.bacc as bacc
    import concourse.mybir as mybir
    import concourse.tile as tile

    f32 = mybir.dt.float32
    nc = bacc.Bacc("TRN2", target_bir_lowering=False, debug=False,
                   num_devices=N_CORES)
    x_t = nc.dram_tensor("x_s", [ROWS, C], f32, kind="ExternalInput")
    u_t = nc.dram_tensor("u_s", [ROWS, C], f32, kind="ExternalInput")
    p_t = nc.dram_tensor("pfull", [P, C], f32, kind="ExternalInput")
    inv_t = nc.dram_tensor("invfull", [P, C], f32, kind="ExternalInput")
    out_t = nc.dram_tensor("out_s", [ROWS, C], f32, kind="ExternalOutput")

    n_grp = ROWS // (P * KGRP)  # 3 groups of 6 interleaved row-tiles
    tail0 = n_grp * P * KGRP  # 2304
    W = KGRP * C  # 4608

    with tile.TileContext(nc) as tc:
        with (
            tc.tile_pool(name="const", bufs=1) as constp,
            tc.tile_pool(name="xin", bufs=2) as xp,
            tc.tile_pool(name="uin", bufs=2) as up,
            tc.tile_pool(name="work", bufs=2) as wp,
            tc.tile_pool(name="tails", bufs=1) as tp,
        ):
            pf = constp.tile([P, C], f32)
            nc.sync.dma_start(pf[:], p_t.ap())
            invf = constp.tile([P, C], f32)
            nc.scalar.dma_start(invf[:], inv_t.ap())
            pfb = pf[:, None, :].broadcast_to([P, KGRP, C])
            invfb = invf[:, None, :].broadcast_to([P, KGRP, C])

            for g in range(n_grp):
                r0 = g * P * KGRP
                r1 = r0 + P * KGRP
                # tile[p, k, c] = x[r0 + k*128 + p, c]
                xs_ap = x_t.ap()[r0:r1, :].rearrange("(k p) c -> p k c", p=P)
                us_ap = u_t.ap()[r0:r1, :].rearrange("(k p) c -> p k c", p=P)
                x_ = xp.tile([P, KGRP, C], f32, tag="x")
                nc.sync.dma_start(x_[:], xs_ap)
                u_ = up.tile([P, KGRP, C], f32, tag="u")
                nc.scalar.dma_start(u_[:], us_ap)

                m_ = wp.tile([P, KGRP, C], f32, tag="m")
                nc.vector.tensor_tensor(m_[:], u_[:], pfb,
                                        op=mybir.AluOpType.is_ge)
                s_ = wp.tile([P, KGRP, C], f32, tag="s")
                nc.vector.tensor_tensor(s_[:], x_[:], invfb,
                                        op=mybir.AluOpType.mult)
                nc.vector.tensor_tensor(m_[:], m_[:], s_[:],
                                        op=mybir.AluOpType.mult)
                nc.sync.dma_start(
                    out_t.ap()[r0:r1, :].rearrange("(k p) c -> p k c", p=P),
                    m_[:])

            # tail rows (2304..2307)
            rows = ROWS - tail0
            x_ = tp.tile([P, C], f32, tag="xt", name="x_tail")
            nc.sync.dma_start(x_[0:rows, :], x_t.ap()[tail0:ROWS, :])
            u_ = tp.tile([P, C], f32, tag="ut", name="u_tail")
            nc.scalar.dma_start(u_[0:rows, :], u_t.ap()[tail0:ROWS, :])
            m_ = tp.tile([P, C], f32, tag="mt", name="m_tail")
            nc.vector.tensor_tensor(m_[0:rows, :], u_[0:rows, :],
                                    pf[0:rows, :], op=mybir.AluOpType.is_ge)
            s_ = tp.tile([P, C], f32, tag="st", name="s_tail")
            nc.vector.tensor_tensor(s_[0:rows, :], x_[0:rows, :],
                                    invf[0:rows, :], op=mybir.AluOpType.mult)
            nc.vector.tensor_tensor(m_[0:rows, :], m_[0:rows, :],
                                    s_[0:rows, :], op=mybir.AluOpType.mult)
            nc.sync.dma_start(out_t.ap()[tail0:ROWS, :], m_[0:rows, :])
    nc.compile()
    return nc


def _get_state():
    if "nc_a" not in _STATE:
        dist_flat, u_perm = _build_consts()
        _STATE["dist_flat"] = dist_flat
        _STATE["u_perm"] = u_perm
        _STATE["nc_a"] = _build_phase_a()
        _STATE["nc_b"] = _build_phase_b()
    return _STATE


def _run_phases(x, attn, trace=False, trace_kwargs=None):
    from concourse.bass_utils import run_bass_kernel_spmd

    st = _get_state()
    dist_flat = st["dist_flat"]
    u_perm = st["u_perm"]

    x = np.ascontiguousarray(x, dtype=np.float32)
    attn = np.ascontiguousarray(attn, dtype=np.float32)

    tkw = dict(trace_kwargs or {})

    # ---- phase A: per-head num/den partial reductions ----
    in_maps_a = []
    for i in range(N_CORES):
        attn_s = attn[i * B_LOC:(i + 1) * B_LOC].reshape(B_LOC * H, NN)
        in_maps_a.append({"attn_s": attn_s, "dist": dist_flat})
    res_a = run_bass_kernel_spmd(st["nc_a"], in_maps_a,
                                 core_ids=list(range(N_CORES)),
                                 trace=trace, **tkw)

    # ---- host: combine partials in float64, compute p ----
    num = np.zeros(H, dtype=np.float64)
    den = np.zeros(H, dtype=np.float64)
    for i in range(N_CORES):
        parts = res_a.results[i]["partials"].astype(np.float64)
        for h in range(H):
            num[h] += parts[:, h * NCH:(h + 1) * NCH].sum()
            den[h] += parts[0, NCH * H + h * NDC:
                            NCH * H + (h + 1) * NDC].sum()
    # straggler element (i=j=576): dist==0 there, denominator only
    den += attn[:, :, N - 1, N - 1].astype(np.float64).sum(axis=0)

    avg = num / den
    p64 = 0.5 / np.exp(5.0 * avg / SIDE)
    p = p64.astype(np.float32)  # [H]
    om = (np.float32(1.0) - p).astype(np.float32)
    inv = (np.float32(1.0) / om).astype(np.float32)

    p_vec = np.repeat(p, Ch).astype(np.float32)  # [C]
    inv_vec = np.repeat(inv, Ch).astype(np.float32)
    pfull = np.ascontiguousarray(np.broadcast_to(p_vec, (P, C)))
    invfull = np.ascontiguousarray(np.broadcast_to(inv_vec, (P, C)))

    # ---- phase B: dropout ----
    in_maps_b = []
    for i in range(N_CORES):
        x_s = x[i * B_LOC:(i + 1) * B_LOC].reshape(ROWS, C)
        u_s = u_perm[i * B_LOC:(i + 1) * B_LOC].reshape(ROWS, C)
        in_maps_b.append({"x_s": x_s, "u_s": u_s,
                          "pfull": pfull, "invfull": invfull})
    res_b = run_bass_kernel_spmd(st["nc_b"], in_maps_b,
                                 core_ids=list(range(N_CORES)),
                                 trace=trace, **tkw)

    out = np.empty((B, N, C), dtype=np.float32)
    for i in range(N_CORES):
        out[i * B_LOC:(i + 1) * B_LOC] = (
            res_b.results[i]["out_s"].reshape(B_LOC, N, C))
    return out, res_a, res_b, dict(p=p, num=num, den=den)


def kernel(x, attn):
    out, _, _, _ = _run_phases(x, attn, trace=False)
    return out
